# revision 2
# baseline (speedup 1.0000x reference)
"""Trainium2 Bass kernel for nn_BatchMuSc (retrieval_knn).

Computes, for Z [96, 256, 128] and cls_tokens [96, 768]:
  - MSM patch anomaly scores: for each image i, for each of its 256 patches,
    the mean of the 28 smallest per-reference-image minimal euclidean
    distances to all other images' patches.
  - img_scores = max over patches; min-max normalize.
  - RsCIN/MMO refinement with W = cls @ cls.T, top-k row masks (k=1,2,3).
  Output: [96] float32.

Strategy (8 NeuronCores, data-parallel over query images):
  - Every core receives the full Z, rolled by -12*core images, so its 12
    query images are always local images 0..11 (static addressing; SPMD).
  - Per core: ZT [128(C), 24576] resident in SBUF (fp32r), distances via
    PSUM-accumulated fp32r matmuls: B = 2*q.z - |z|^2 (rank-1 adds -|z|^2),
    grouped max-reduce over each reference image's 256 patches
    (max B = -min d2), then top-28-smallest via max8/match_replace.
  - img_scores are AllGathered across cores; every core redundantly runs the
    tiny MMO refinement; core 0's output is returned.
"""
import os
import sys
import types

import numpy as np

for _p in ("/opt/trn_rl_repo",):
    if _p not in sys.path and os.path.isdir(_p):
        sys.path.insert(0, _p)

# The axon NTFF profile hook module is absent in this environment; stub it so
# run_bass_kernel_spmd can import it (only needed for trace=True).
try:  # pragma: no cover
    import antenv.axon_hooks  # noqa: F401
except Exception:  # pragma: no cover
    _m = types.ModuleType("antenv.axon_hooks")
    _m.get_axon_ntff_profile_hook = lambda: None
    sys.modules["antenv.axon_hooks"] = _m

import concourse.bacc as bacc
import concourse.bass_isa as bass_isa
import concourse.mybir as mybir
from concourse import bass_utils
from concourse.masks import make_identity
from concourse.tile import TileContext

F32 = mybir.dt.float32
F32R = mybir.dt.float32r
FP16 = mybir.dt.float16
AX = mybir.AxisListType.X
OP = mybir.AluOpType
ACTF = mybir.ActivationFunctionType

N, L, C, DC = 96, 256, 128, 768
NCORES = 8
IPC = N // NCORES          # 12 query images per core
NL = N * L                 # 24576 total patches
NT = NL // 128             # 192 transpose tiles
NS = NL // 512             # 48 stripes of 512 patches (2 images each)
NQ = NS // 4               # 12 quads of 4 stripes (8 images each)
KTOP = 28                  # int((N-1)*0.3) smallest distances averaged
EPS = 1e-12
NEG = -3.4e38

# Per-quad reduction path: D = DVE reduce from PSUM;
# A = ACT copy to SBUF + DVE reduce; G = ACT copy + GPSIMD pairwise fold +
# DVE short reduce.  Tunable (len NQ).
# Per-quad paths:
#  D = DVE reduce direct from PSUM (needs rank-1)
#  A = ACT copy to SBUF + DVE flat reduce (needs rank-1)
#  R = rank-1 on PE + ACT copy + DVE pairwise-max tree
#  C = no rank-1: ACT copy + DVE tensor-add of -|z|^2 + pairwise-max tree
QUAD_PATHS = os.environ.get("BMS_QUAD_PATHS", "RRRDRRRDRRRD")
assert len(QUAD_PATHS) == NQ and set(QUAD_PATHS) <= set("DARC")
MM_DT = os.environ.get("BMS_MM_DT", "fp16")       # matmul operand dtype
RED_DT = os.environ.get("BMS_RED_DT", "fp16")     # copy/reduce dtype


def build(
    quad_paths: str = QUAD_PATHS,
    repeat_main: int = 1,
    n_cores: int = NCORES,
    stop: str = "full",
    dummy: bool = False,
    ablate: str = "",
    mm_dt: str = None,
    red_dt: str = None,
):
    mm_dt = MM_DT if mm_dt is None else mm_dt
    red_dt = RED_DT if red_dt is None else red_dt
    MDT = {"f32r": F32R, "fp16": FP16}[mm_dt]
    RDT = {"f32": F32, "fp16": FP16}[red_dt]
    nc = bacc.Bacc(
        "TRN2",
        target_bir_lowering=False,
        debug=False,
        enable_asserts=False,
        num_devices=n_cores,
    )
    Z = None if dummy else nc.dram_tensor("Z", [N, L, C], FP16, kind="ExternalInput")
    cls = nc.dram_tensor("cls_tokens", [N, DC], F32, kind="ExternalInput")
    out = nc.dram_tensor("out", [N], F32, kind="ExternalOutput")
    cc_in = nc.dram_tensor("cc_in", [IPC], F32, kind="Internal")
    cc_out = nc.dram_tensor("cc_out", [N], F32, kind="Internal", addr_space="Shared")

    stages = ["p0", "p0b", "p1", "p2", "full"]
    sidx = stages.index(stop)
    with TileContext(nc) as tc:
        with tc.tile_pool(name="persist", bufs=1) as pers:
            ident = pers.tile([128, 128], F32)
            make_identity(nc, ident)
            ones_f = pers.tile([128, 128], F32)
            nc.vector.memset(ones_f, 1.0)
            ones_r = pers.tile([128, 128], MDT)
            nc.vector.tensor_copy(ones_r, ones_f)
            epsb = pers.tile([128, 1], F32)
            nc.vector.memset(epsb, EPS)
            oinv_f = pers.tile([128, 128], F32)
            nc.vector.memset(oinv_f, 1.0 / 128.0)
            ones_inv = pers.tile([128, 128], MDT)
            nc.vector.tensor_copy(ones_inv, oinv_f)

            ZT = pers.tile([128, NL], MDT)           # channels x patches
            sq_q = pers.tile([128, 2 * IPC], F32)    # |z|^2 of local queries
            # -|z_p|^2 packed for rank-1 matmul rhs reads. Matmul operands
            # must start at partition 0/32/64, so stripes live on exactly
            # those three rows: row 32*(s//16), columns 512*(s%16).
            nsq = pers.tile([65, 16 * 512], MDT)
            # negated |z_p|^2 replicated across partitions; used both as the
            # rank-128 matmul rhs (adds -|z|^2 into PSUM at full matmul speed:
            # lhsT is the constant 1/128 matrix, and summing 128 identical
            # fp16 values of v/128 in fp32 PSUM reconstructs v exactly) and
            # for C-path DVE adds.
            nsq_rep = pers.tile([128, NL], MDT)
            score_all = pers.tile([128, 2 * IPC], F32)
            simg = pers.tile([1, N], F32)

            # ---- Phase 0: load Z, build ZT (transposed, fp32r), query norms
            if dummy:
                if mm_dt == "f32r":
                    nc.vector.memset(ZT.bitcast(F32), 0.5)
                    nc.vector.memset(nsq.bitcast(F32), -32.0)
                else:
                    nc.vector.memset(ZT, 0.5)
                    nc.vector.memset(nsq, -32.0)
                nc.vector.memset(sq_q, float(C) * 0.25)
                nc.vector.memset(nsq_rep, -32.0)
            else:
              # ZT built by DMA xbar transposes STRAIGHT from DRAM fp16
              # (one per 12-tile batch); small staged loads only for the
              # query-norm squares (patch index must sit on partitions there).
              Zf = Z.ap().rearrange("n l c -> (n l) c")
              TBATCH = 12
              with (
                tc.tile_pool(name="stage", bufs=2) as stage,
                tc.tile_pool(name="sqscr", bufs=2) as sqscr,
              ):
                assert MDT == FP16, "fast P0 loads Z as fp16"
                for b in range(NT // TBATCH):
                    nc.sync.dma_start_transpose(
                        ZT[:, 128 * TBATCH * b : 128 * TBATCH * (b + 1)],
                        Zf[128 * TBATCH * b : 128 * TBATCH * (b + 1), :],
                    )
                for b in range(2 * IPC // TBATCH):
                    bt = stage.tile([128, TBATCH, C], FP16, tag=f"b{b % 2}")
                    nc.sync.dma_start(
                        bt,
                        Zf[
                            128 * TBATCH * b : 128 * TBATCH * (b + 1), :
                        ].rearrange("(t p) c -> p t c", p=128),
                    )
                    for tt in range(TBATCH):
                        t = TBATCH * b + tt
                        dm = sqscr.tile([128, C], F32, tag="dm")
                        nc.scalar.activation(
                            dm, bt[:, tt, :], ACTF.Square,
                            accum_out=sq_q[:, t : t + 1],
                        )

            # ---- Phase 0b: negated patch norms -|z_p|^2 in rank-1 layout
            if sidx >= 1:
              with (
                tc.tile_pool(name="z2p", bufs=3) as z2p,
                tc.tile_pool(name="sqpsum", bufs=4, space="PSUM") as sqp,
              ):
                for s in range(NS):
                    z2 = z2p.tile([128, 512], MDT, tag="z2")
                    nc.vector.tensor_mul(
                        z2, ZT[:, 512 * s : 512 * (s + 1)], ZT[:, 512 * s : 512 * (s + 1)]
                    )
                    psq = sqp.tile([128, 512], F32, tag="psq")
                    nc.tensor.matmul(psq, lhsT=ones_r, rhs=z2, start=True, stop=True)
                    # every psq row holds the same column sums; copy from the
                    # partition row matching nsq's layout
                    row = 32 * (s // 16)
                    off = 512 * (s % 16)
                    nc.scalar.mul(
                        nsq[row : row + 1, off : off + 512],
                        psq[row : row + 1, :],
                        -1.0,
                    )
                    nc.scalar.mul(
                        nsq_rep[:, 512 * s : 512 * (s + 1)], psq, -1.0
                    )

            # ---- Phase 1: distances + per-image minima + top-28 means
            if sidx >= 2:
              with (
                tc.tile_pool(name="q2p", bufs=2) as q2p,
                tc.tile_pool(name="quadp", bufs=2, space="PSUM") as quadp,
                tc.tile_pool(name="cpp", bufs=3) as cpp,
                tc.tile_pool(name="foldp", bufs=2) as foldp,
                tc.tile_pool(name="maxbp", bufs=2) as maxbp,
                tc.tile_pool(name="smallp", bufs=2) as smallp,
                tc.tile_pool(name="treep", bufs=2) as treep,
                tc.tile_pool(name="bsubp", bufs=2) as bsubp,
              ):
                def tree_fold(srct, mslice):
                    v = srct.rearrange("p (g two x) -> p g two x", g=8, two=2)
                    f = treep.tile([128, 8, 128], RDT, tag="t128")
                    nc.vector.tensor_tensor(f, v[:, :, 0, :], v[:, :, 1, :], op=OP.max)
                    for w in (64, 32, 16, 8):
                        nxt = treep.tile([128, 8, w], RDT, tag=f"t{w}")
                        vv = f.rearrange("p g (two x) -> p g two x", two=2)
                        nc.vector.tensor_tensor(
                            nxt, vv[:, :, 0, :], vv[:, :, 1, :], op=OP.max
                        )
                        f = nxt
                    nc.vector.tensor_reduce(mslice, f, axis=AX, op=OP.max)

                from contextlib import nullcontext
                loop_cm = (
                    tc.For_i(0, repeat_main, 1) if repeat_main > 1 else nullcontext()
                )
                with loop_cm:
                    for i in range(IPC):
                        q2 = q2p.tile([128, L], MDT, tag="q2")
                        nc.scalar.mul(q2, ZT[:, L * i : L * (i + 1)], 2.0)
                        for h in range(2):
                            lhsT = q2[:, 128 * h : 128 * (h + 1)]
                            maxB = maxbp.tile([128, N], RDT, tag="maxB")
                            for qd in range(NQ):
                                path = quad_paths[qd]
                                need_r1 = path in "DAR"
                                quad = quadp.tile([128, 4, 512], F32, tag="quad")
                                if need_r1:
                                    for j in range(4):
                                        s = 4 * qd + j
                                        nc.tensor.matmul(
                                            quad[:, j, :],
                                            lhsT=ones_inv,
                                            rhs=nsq_rep[
                                                :, 512 * s : 512 * (s + 1)
                                            ],
                                            start=True,
                                            stop=False,
                                        )
                                for j in range(4):
                                    s = 4 * qd + j
                                    nc.tensor.matmul(
                                        quad[:, j, :],
                                        lhsT=lhsT,
                                        rhs=ZT[:, 512 * s : 512 * (s + 1)],
                                        start=not need_r1,
                                        stop=True,
                                    )
                                mslice = maxB[:, 8 * qd : 8 * qd + 8]
                                qflat = quad.rearrange("p f x -> p (f x)")
                                if ablate == "nored":
                                    if qd == 0:
                                        nc.vector.memset(maxB, -100.0)
                                elif path == "D":
                                    nc.vector.tensor_reduce(
                                        mslice,
                                        qflat.rearrange("p (g x) -> p g x", g=8),
                                        axis=AX,
                                        op=OP.max,
                                    )
                                elif path == "A":
                                    cp = cpp.tile([128, 2048], RDT, tag="cp")
                                    nc.scalar.copy(cp, qflat)
                                    nc.vector.tensor_reduce(
                                        mslice,
                                        cp.rearrange("p (g x) -> p g x", g=8),
                                        axis=AX,
                                        op=OP.max,
                                    )
                                else:  # R or C: ACT copy + (C: add -|z|^2) + tree
                                    cp = cpp.tile([128, 2048], RDT, tag="cp")
                                    nc.scalar.copy(cp, qflat)
                                    if path == "C":
                                        b2 = bsubp.tile(
                                            [128, 2048], RDT, tag="bsub"
                                        )
                                        nc.vector.tensor_add(
                                            b2,
                                            cp,
                                            nsq_rep[
                                                :, 2048 * qd : 2048 * (qd + 1)
                                            ],
                                        )
                                        srct = b2
                                    else:
                                        srct = cp
                                    tree_fold(srct, mslice)
                            # finalize (i, h): x = min(maxB - sq_q, 0) = -d2c
                            col = 2 * i + h
                            if ablate == "nofin":
                                nc.vector.tensor_scalar(
                                    score_all[:, col : col + 1],
                                    maxB[:, 0:1],
                                    1.0,
                                    None,
                                    op0=OP.mult,
                                )
                                continue
                            x = smallp.tile([128, N], F32, tag="x")
                            nc.vector.tensor_scalar(
                                x,
                                maxB,
                                sq_q[:, col : col + 1],
                                0.0,
                                op0=OP.subtract,
                                op1=OP.min,
                            )
                            nc.vector.memset(x[:, i : i + 1], NEG)
                            b8 = smallp.tile([128, 32], F32, tag="b8")
                            for r in range(4):
                                nc.vector.max(b8[:, 8 * r : 8 * r + 8], x)
                                if r < 3:
                                    nc.vector.match_replace(
                                        x,
                                        in_to_replace=b8[:, 8 * r : 8 * r + 8],
                                        in_values=x,
                                        imm_value=NEG,
                                    )
                            sv = smallp.tile([128, KTOP], F32, tag="sv")
                            nc.scalar.activation(
                                sv, b8[:, 0:KTOP], ACTF.Sqrt, bias=epsb, scale=-1.0
                            )
                            s28 = smallp.tile([128, 1], F32, tag="s28")
                            nc.vector.reduce_sum(s28, sv, axis=AX)
                            nc.vector.tensor_scalar(
                                score_all[:, col : col + 1],
                                s28,
                                1.0 / KTOP,
                                None,
                                op0=OP.mult,
                            )

            # ---- Phase 2: image scores + AllGather
            if sidx >= 3:
              with tc.tile_pool(name="p2", bufs=1) as p2:
                red = p2.tile([128, 2 * IPC], F32)
                nc.gpsimd.partition_all_reduce(
                    red, score_all, channels=128, reduce_op=bass_isa.ReduceOp.max
                )
                img12 = p2.tile([1, IPC], F32)
                nc.vector.tensor_reduce(
                    img12,
                    red[0:1, :].rearrange("p (i h) -> p i h", h=2),
                    axis=AX,
                    op=OP.max,
                )
                nc.sync.dma_start(cc_in.ap(), img12)
                nc.gpsimd.collective_compute(
                    "AllGather",
                    OP.bypass,
                    replica_groups=[list(range(NCORES))],
                    ins=[cc_in.ap()],
                    outs=[cc_out.ap()],
                )
                nc.sync.dma_start(simg, cc_out.ap())

            # ---- Phase 3: RsCIN / MMO (redundant on every core)
            if sidx >= 4:
              with (
                tc.tile_pool(name="p3", bufs=1) as p3,
                tc.tile_pool(name="p3psum", bufs=2, space="PSUM") as p3p,
              ):
                mn = p3.tile([1, 1], F32)
                mx = p3.tile([1, 1], F32)
                nc.vector.tensor_reduce(mn, simg, axis=AX, op=OP.min)
                nc.vector.tensor_reduce(mx, simg, axis=AX, op=OP.max)
                rngv = p3.tile([1, 1], F32)
                nc.vector.tensor_sub(rngv, mx, mn)
                rcp = p3.tile([1, 1], F32)
                nc.vector.reciprocal(rcp, rngv)
                s_norm = p3.tile([1, N], F32)
                nc.vector.tensor_scalar(
                    s_norm, simg, mn, rcp, op0=OP.subtract, op1=OP.mult
                )
                s_rep = p3.tile([N, N], F32)
                nc.gpsimd.partition_broadcast(s_rep, s_norm, channels=N)

                cls_sb = p3.tile([N, DC], F32)
                nc.sync.dma_start(cls_sb, cls.ap())
                clsT = p3.tile([128, DC // 128, N], F32)
                for d in range(DC // 128):
                    pt = p3p.tile([128, N], F32, tag="pt3")
                    nc.tensor.transpose(
                        pt, cls_sb[:, 128 * d : 128 * (d + 1)], ident[0:N, 0:N]
                    )
                    nc.scalar.copy(clsT[:, d, :], pt)
                Wp = p3p.tile([N, N], F32, tag="Wp")
                for d in range(DC // 128):
                    nc.tensor.matmul(
                        Wp,
                        lhsT=clsT[:, d, :],
                        rhs=clsT[:, d, :],
                        start=(d == 0),
                        stop=(d == DC // 128 - 1),
                    )
                W = p3.tile([N, N], F32)
                nc.scalar.copy(W, Wp)
                m8w = p3.tile([N, 8], F32)
                nc.vector.max(m8w, W)
                acc = p3.tile([N, 1], F32)
                nc.vector.memset(acc, 0.0)
                Wm = p3.tile([N, N], F32)
                Pk = p3.tile([N, N], F32)
                for k in (1, 2, 3):
                    rs = p3.tile([N, 1], F32, tag=f"rs{k}")
                    nc.vector.scalar_tensor_tensor(
                        out=Wm,
                        in0=W,
                        scalar=m8w[:, k - 1 : k],
                        in1=W,
                        op0=OP.is_ge,
                        op1=OP.mult,
                        accum_out=rs,
                    )
                    rck = p3.tile([N, 1], F32, tag=f"rck{k}")
                    nc.vector.reciprocal(rck, rs)
                    Sk = p3.tile([N, 1], F32, tag=f"Sk{k}")
                    nc.vector.tensor_mul(Pk, Wm, s_rep)
                    nc.vector.reduce_sum(Sk, Pk, axis=AX)
                    term = p3.tile([N, 1], F32, tag=f"term{k}")
                    nc.vector.tensor_scalar(term, Sk, rck, None, op0=OP.mult)
                    nc.vector.tensor_add(acc, acc, term)
                out_sb = p3.tile([N, 1], F32)
                nc.vector.tensor_scalar(
                    out_sb, acc, 1.0 / 3.0, None, op0=OP.mult
                )
                nc.sync.dma_start(out.ap(), out_sb)
            if sidx < 4:
                with tc.tile_pool(name="dbg", bufs=1) as dbg:
                    dt_ = dbg.tile([1, N], F32)
                    src_ap = score_all[0:1, 0:24] if sidx >= 2 else ZT[0:1, 0:24]
                    nc.vector.tensor_scalar(
                        dt_[:, 0:24], src_ap.bitcast(F32), 1.0, None, op0=OP.mult
                    )
                    nc.vector.memset(dt_[:, 24:N], 0.0)
                    nc.sync.dma_start(out.ap(), dt_)

    nc.finalize()
    return nc


_CACHE: dict = {}


def _get_nc():
    key = (QUAD_PATHS,)
    if key not in _CACHE:
        _CACHE[key] = build(QUAD_PATHS)
    return _CACHE[key]


def kernel(Z: np.ndarray, cls_tokens: np.ndarray) -> np.ndarray:
    assert Z.shape == (N, L, C) and cls_tokens.shape == (N, DC)
    Z = np.asarray(Z, dtype=np.float32).astype(np.float16)
    cls_tokens = np.ascontiguousarray(cls_tokens, dtype=np.float32)
    nc = _get_nc()
    in_maps = [
        {"Z": np.ascontiguousarray(np.roll(Z, -IPC * c, axis=0)), "cls_tokens": cls_tokens}
        for c in range(NCORES)
    ]
    res = bass_utils.run_bass_kernel_spmd(nc, in_maps, core_ids=list(range(NCORES)))
    return np.asarray(res.results[0]["out"], dtype=np.float32)


if __name__ == "__main__":
    rng = np.random.default_rng(0)
    Zv = rng.standard_normal((N, L, C), dtype=np.float32)
    cv = rng.standard_normal((N, DC), dtype=np.float32)
    print(kernel(Zv, cv)[:8])



# revision 3
# speedup vs baseline: 1.4884x; 1.4884x over previous
"""Trainium2 Bass kernel for nn_BatchMuSc (retrieval_knn) — v2.

Computes, for Z [96, 256, 128] and cls_tokens [96, 768]:
  - MSM patch anomaly scores: for each image i, for each of its 256 patches,
    the mean of the 28 smallest per-reference-image minimal euclidean
    distances to all other images' patches.
  - img_scores = max over patches; min-max normalize.
  - RsCIN/MMO refinement with W = cls @ cls.T, top-k row masks (k=1,2,3).
  Output: [96] float32.

Strategy (8 NeuronCores, data-parallel over query images):
  - Every core receives the full Z, rolled by -12*core images, so its 12
    query images are always local images 0..11 (static addressing; SPMD).
  - ZT [128(C), 24576] fp16 resident in SBUF. B' = q.z - |z|^2/2 so that
    d2 = |q|^2 - 2 B'; per-image max of B' gives -min d2 / 2.
  - Persistent-PSUM delta chains: ref patches are split into 16 groups of
    3 stripes (1536 patches, 6 images). Per group, PSUM is initialized
    once with -|z|^2/2 (matmul with constant -1/2 lhsT and rhs=ZT*ZT) and
    the 24 query tiles are then applied incrementally: step k accumulates
    lhsT = (q_k - q_{k-1}) so no per-step norm matmul is needed. Re-anchored
    with a fresh init + full q every ANCHOR steps to bound fp16 drift.
  - Per-image max reduce of each group [128, 6*256] -> [128, 6] is split
    across ACT (PSUM->fp16 copy), Pool (pairwise max level-0), and DVE
    (fp16 max tree), per-group tunable.
  - Finalize per (i,h): top-32 via max8/match_replace, then a single ACT
    Sqrt(scale=-2, bias=|q|^2) with accum_out summing the top-28 (the 1/28
    mean and any positive scale cancel in min-max normalization).
  - img_scores are AllGathered across cores; every core redundantly runs the
    tiny MMO refinement; core 0's output is returned.
"""
import os
import sys
import types

import numpy as np

for _p in ("/opt/trn_rl_repo",):
    if _p not in sys.path and os.path.isdir(_p):
        sys.path.insert(0, _p)

# The axon NTFF profile hook module is absent in this environment; stub it so
# run_bass_kernel_spmd can import it (only needed for trace=True).
try:  # pragma: no cover
    import antenv.axon_hooks  # noqa: F401
except Exception:  # pragma: no cover
    _m = types.ModuleType("antenv.axon_hooks")
    _m.get_axon_ntff_profile_hook = lambda: None
    sys.modules["antenv.axon_hooks"] = _m

import concourse.bacc as bacc
import concourse.bass_isa as bass_isa
import concourse.mybir as mybir
from concourse import bass_utils
from concourse.masks import make_identity
from concourse.tile import TileContext

F32 = mybir.dt.float32
FP16 = mybir.dt.float16
AX = mybir.AxisListType.X
OP = mybir.AluOpType
ACTF = mybir.ActivationFunctionType

N, L, C, DC = 96, 256, 128, 768
NCORES = 8
IPC = N // NCORES          # 12 query images per core
NL = N * L                 # 24576 total patches
NT = NL // 128             # 192 transpose tiles
NS = NL // 512             # 48 stripes of 512 patches (2 images each)
GS = 3                     # stripes per PSUM chain group (3 banks)
NG = NS // GS              # 16 groups of 6 images
NK = 2 * IPC               # 24 (image, half) steps
KTOP = 28                  # int((N-1)*0.3) smallest distances averaged
EPS = 1e-12
NEG = -3.4e38

# Per-group reduce path (len NG):
#  A = ACT copy PSUM->fp16, DVE max tree
#  P = Pool pairwise-max level0 from PSUM, DVE tree from 768
#  Q = Pool level0+level1, DVE tree from 384
#  D = DVE level0 from PSUM, DVE tree from 768
PATHS = os.environ.get("BMS2_PATHS", "AGAGAGAGAGAGAGAG")
ANCHOR = int(os.environ.get("BMS2_ANCHOR", "12"))


def build(
    paths: str = PATHS,
    anchor: int = ANCHOR,
    n_cores: int = NCORES,
    stop: str = "full",
    split: bool = False,       # split touches to release the PSUM WAR early
):
    assert len(paths) == NG and set(paths) <= set("AG")
    nc = bacc.Bacc(
        "TRN2",
        target_bir_lowering=False,
        debug=False,
        enable_asserts=False,
        num_devices=n_cores,
    )
    Z = nc.dram_tensor("Z", [N, L, C], FP16, kind="ExternalInput")
    cls = nc.dram_tensor("cls_tokens", [N, DC], F32, kind="ExternalInput")
    out = nc.dram_tensor("out", [N], F32, kind="ExternalOutput")
    cc_in = nc.dram_tensor("cc_in", [IPC], F32, kind="Internal")
    cc_out = nc.dram_tensor("cc_out", [N], F32, kind="Internal", addr_space="Shared")

    stages = ["p0", "p1", "p2", "full"]
    sidx = stages.index(stop)
    with TileContext(nc) as tc:
        with tc.tile_pool(name="persist", bufs=1) as pers:
            ident = pers.tile([128, 128], F32)
            make_identity(nc, ident)
            neghalf_f = pers.tile([128, 128], F32)
            nc.vector.memset(neghalf_f, -0.5)
            neghalf = pers.tile([128, 128], FP16)
            nc.vector.tensor_copy(neghalf, neghalf_f)
            epsb = pers.tile([128, 1], F32)
            nc.vector.memset(epsb, EPS)

            ZT = pers.tile([128, NL], FP16)          # channels x patches
            q2d = pers.tile([128, (NK - 1) * 128], FP16)  # query deltas
            sq_q = pers.tile([128, NK], F32)         # |q|^2 per (i,h)
            mB = pers.tile([128, NK, N], FP16)       # per-image max of B'
            score_all = pers.tile([128, NK], F32)
            simg = pers.tile([1, N], F32)

            # ---- Phases 0+1 interleaved: stream Z in per-round tile
            # batches while the delta chains run two rounds behind.
            Zf = Z.ap().rearrange("n l c -> (n l) c")
            TB = 12                      # tiles per DMA batch (2 batches/round)
            NB = NT // TB
            with (
                tc.tile_pool(name="zstage", bufs=4) as stage,
                tc.tile_pool(name="sqscr", bufs=2) as sqscr,
                tc.tile_pool(name="chains", bufs=1, space="PSUM") as chp,
                tc.tile_pool(name="z2p", bufs=1) as z2p,
                tc.tile_pool(name="cpp", bufs=bufs) as cpp,
                tc.tile_pool(name="treep", bufs=bufs) as treep,
                tc.tile_pool(name="finp", bufs=4) as finp,
            ):
                batches = {}

                def emit_tile(t):
                    b = t // TB
                    if b not in batches:
                        bt = stage.tile([128, TB, C], FP16, tag=f"b{b % 4}")
                        nc.sync.dma_start(
                            bt,
                            Zf[128 * TB * b : 128 * TB * (b + 1), :].rearrange(
                                "(t p) c -> p t c", p=128
                            ),
                        )
                        batches[b] = bt
                    st = batches[b][:, t % TB, :]
                    nc.sync.dma_start_transpose(
                        ZT[:, 128 * t : 128 * (t + 1)], st
                    )
                    if t < NK:
                        dm = sqscr.tile([128, C], F32, tag="dm")
                        nc.scalar.activation(
                            dm, st, ACTF.Square, accum_out=sq_q[:, t : t + 1]
                        )

                def reduce_one(ch, r, k, idx, path, hp=False):
                    from contextlib import nullcontext
                    hpcm = tc.high_priority() if hp else nullcontext()
                    # Per-image max for one chain [128, 1536] PSUM -> mB[..6].
                    # Legal engine set: ACT copy (1 PSUM input) + DVE/Pool fp16
                    # tree (A/B/C = 0/1/2 Pool levels), or a single DVE flat
                    # grouped tensor_reduce from PSUM (G). GPSIMD cannot read
                    # PSUM, and DVE tensor_tensor allows only one PSUM input.
                    g = 2 * r + idx
                    mslice = mB[:, k, 6 * g : 6 * g + 6]
                    if path == "G":
                      with hpcm:
                        if split:  # noqa
                            pass
                            for h in range(2):
                                nc.vector.tensor_reduce(
                                    mslice[:, 3 * h : 3 * h + 3],
                                    ch[:, 768 * h : 768 * (h + 1)].rearrange(
                                        "p (g x) -> p g x", g=3
                                    ),
                                    axis=AX,
                                    op=OP.max,
                                )
                        else:
                            nc.vector.tensor_reduce(
                                mslice,
                                ch.rearrange("p (g x) -> p g x", g=6),
                                axis=AX,
                                op=OP.max,
                            )
                      return
                    # Pool/GPSIMD cannot run TensorTensor at all (ucode ops
                    # only), so the whole tree stays on DVE.
                    cp = cpp.tile([128, GS * 512], FP16, tag=f"cp{idx}")
                    with hpcm:
                        if split:
                            nc.scalar.copy(cp[:, 0:768], ch[:, 0:768])
                            nc.scalar.copy(cp[:, 768:1536], ch[:, 768:1536])
                        else:
                            nc.scalar.copy(cp, ch)
                    cv = cp.rearrange("p (g two x) -> p g two x", g=6, two=2)
                    t768 = treep.tile([128, 6, 128], FP16, tag=f"t768{idx}")
                    nc.vector.tensor_tensor(
                        t768, cv[:, :, 0, :], cv[:, :, 1, :], op=OP.max
                    )
                    cur = t768
                    for w in (64, 32, 16):
                        nxt = treep.tile([128, 6, w], FP16, tag=f"t{w}{idx}")
                        cc = cur.rearrange("p g (two x) -> p g two x", two=2)
                        nc.vector.tensor_tensor(
                            nxt, cc[:, :, 0, :], cc[:, :, 1, :], op=OP.max
                        )
                        cur = nxt
                    nc.vector.tensor_reduce(mslice, cur, axis=AX, op=OP.max)

                # prologue: tiles for rounds 0 and 1, then query deltas
                for t in range(48):
                    emit_tile(t)
                nc.vector.tensor_sub(
                    q2d, ZT[:, 128 : NK * 128], ZT[:, 0 : (NK - 1) * 128]
                )

                if sidx >= 1:
                  for r in range(NG // 2):
                    gpair = (2 * r, 2 * r + 1)
                    chs = {}
                    for g in gpair:
                        ch_t = chp.tile([128, GS * 512], F32, tag=f"c{g % 2}")
                        z2_t = z2p.tile([128, GS * 512], FP16, tag=f"z2{g % 2}")
                        chs[g] = (ch_t, z2_t)
                        zg = ZT[:, 1536 * g : 1536 * (g + 1)]
                        nc.vector.tensor_mul(z2_t, zg, zg)
                    pre = [24 * (r + 2) + u for u in range(24)] if r + 2 < NG // 2 else []
                    for k in range(NK):
                        anchored = k % anchor == 0
                        if anchored:
                            lhsT = ZT[:, 128 * k : 128 * (k + 1)]
                        else:
                            lhsT = q2d[:, 128 * (k - 1) : 128 * k]
                        for g in gpair:
                            ch, z2_t = chs[g]
                            if anchored:
                                for j in range(GS):
                                    nc.tensor.matmul(
                                        ch[:, 512 * j : 512 * (j + 1)],
                                        lhsT=neghalf,
                                        rhs=z2_t[:, 512 * j : 512 * (j + 1)],
                                        start=True,
                                        stop=False,
                                    )
                            for j in range(GS):
                                s = GS * g + j
                                nc.tensor.matmul(
                                    ch[:, 512 * j : 512 * (j + 1)],
                                    lhsT=lhsT,
                                    rhs=ZT[:, 512 * s : 512 * (s + 1)],
                                    start=False,
                                    stop=True,
                                    skip_group_check=not anchored,
                                )
                        for idx, g in enumerate(gpair):
                            reduce_one(chs[g][0], r, k, idx, paths[g], hp=hp)
                        for t in pre[3 * k : 3 * k + 3]:
                            emit_tile(t)

                  # ---- finalize per (i, h): top-28 mean (scaled by 28)
                  # Selection runs on fp16 mB directly (d2c = sq - 2*mB is
                  # monotone in mB); the ACT Sqrt applies scale/bias and
                  # accum-sums the 28 selected values.
                  for k in range(NK):
                    i = k // 2
                    xm = mB[:, k, :]
                    nc.vector.memset(xm[:, i : i + 1], -60000.0)
                    b8 = finp.tile([128, 32], FP16, tag="b8")
                    for rr in range(4):
                        nc.vector.max(b8[:, 8 * rr : 8 * rr + 8], xm)
                        if rr < 3:
                            nc.vector.match_replace(
                                xm,
                                in_to_replace=b8[:, 8 * rr : 8 * rr + 8],
                                in_values=xm,
                                imm_value=-60000.0,
                            )
                    sv = finp.tile([128, KTOP], FP16, tag="sv")
                    nc.scalar.activation(
                        sv,
                        b8[:, 0:KTOP],
                        ACTF.Sqrt,
                        bias=sq_q[:, k : k + 1],
                        scale=-2.0,
                        accum_out=score_all[:, k : k + 1],
                    )

            # ---- Phase 2: image scores + AllGather
            if sidx >= 2:
              with tc.tile_pool(name="p2", bufs=1) as p2:
                red = p2.tile([128, NK], F32)
                nc.gpsimd.partition_all_reduce(
                    red, score_all, channels=128, reduce_op=bass_isa.ReduceOp.max
                )
                img12 = p2.tile([1, IPC], F32)
                nc.vector.tensor_reduce(
                    img12,
                    red[0:1, :].rearrange("p (i h) -> p i h", h=2),
                    axis=AX,
                    op=OP.max,
                )
                nc.sync.dma_start(cc_in.ap(), img12)
                nc.gpsimd.collective_compute(
                    "AllGather",
                    OP.bypass,
                    replica_groups=[list(range(NCORES))],
                    ins=[cc_in.ap()],
                    outs=[cc_out.ap()],
                )
                nc.sync.dma_start(simg, cc_out.ap())

            # ---- Phase 3: RsCIN / MMO (redundant on every core)
            if sidx >= 3:
              with (
                tc.tile_pool(name="p3", bufs=1) as p3,
                tc.tile_pool(name="p3psum", bufs=2, space="PSUM") as p3p,
              ):
                mn = p3.tile([1, 1], F32)
                mx = p3.tile([1, 1], F32)
                nc.vector.tensor_reduce(mn, simg, axis=AX, op=OP.min)
                nc.vector.tensor_reduce(mx, simg, axis=AX, op=OP.max)
                rngv = p3.tile([1, 1], F32)
                nc.vector.tensor_sub(rngv, mx, mn)
                rcp = p3.tile([1, 1], F32)
                nc.vector.reciprocal(rcp, rngv)
                s_norm = p3.tile([1, N], F32)
                nc.vector.tensor_scalar(
                    s_norm, simg, mn, rcp, op0=OP.subtract, op1=OP.mult
                )
                s_rep = p3.tile([N, N], F32)
                nc.gpsimd.partition_broadcast(s_rep, s_norm, channels=N)

                cls_sb = p3.tile([N, DC], F32)
                nc.sync.dma_start(cls_sb, cls.ap())
                clsT = p3.tile([128, DC // 128, N], F32)
                for d in range(DC // 128):
                    pt = p3p.tile([128, N], F32, tag="pt3")
                    nc.tensor.transpose(
                        pt, cls_sb[:, 128 * d : 128 * (d + 1)], ident[0:N, 0:N]
                    )
                    nc.scalar.copy(clsT[:, d, :], pt)
                Wp = p3p.tile([N, N], F32, tag="Wp")
                for d in range(DC // 128):
                    nc.tensor.matmul(
                        Wp,
                        lhsT=clsT[:, d, :],
                        rhs=clsT[:, d, :],
                        start=(d == 0),
                        stop=(d == DC // 128 - 1),
                    )
                W = p3.tile([N, N], F32)
                nc.scalar.copy(W, Wp)
                m8w = p3.tile([N, 8], F32)
                nc.vector.max(m8w, W)
                acc = p3.tile([N, 1], F32)
                nc.vector.memset(acc, 0.0)
                Wm = p3.tile([N, N], F32)
                Pk = p3.tile([N, N], F32)
                for kk in (1, 2, 3):
                    rs = p3.tile([N, 1], F32, tag=f"rs{kk}")
                    nc.vector.scalar_tensor_tensor(
                        out=Wm,
                        in0=W,
                        scalar=m8w[:, kk - 1 : kk],
                        in1=W,
                        op0=OP.is_ge,
                        op1=OP.mult,
                        accum_out=rs,
                    )
                    rck = p3.tile([N, 1], F32, tag=f"rck{kk}")
                    nc.vector.reciprocal(rck, rs)
                    Sk = p3.tile([N, 1], F32, tag=f"Sk{kk}")
                    nc.vector.tensor_mul(Pk, Wm, s_rep)
                    nc.vector.reduce_sum(Sk, Pk, axis=AX)
                    term = p3.tile([N, 1], F32, tag=f"term{kk}")
                    nc.vector.tensor_scalar(term, Sk, rck, None, op0=OP.mult)
                    nc.vector.tensor_add(acc, acc, term)
                out_sb = p3.tile([N, 1], F32)
                nc.vector.tensor_scalar(
                    out_sb, acc, 1.0 / 3.0, None, op0=OP.mult
                )
                nc.sync.dma_start(out.ap(), out_sb)
            if sidx < 3:
                with tc.tile_pool(name="dbg", bufs=1) as dbg:
                    dt_ = dbg.tile([1, N], F32)
                    src_ap = score_all[0:1, 0:NK] if sidx >= 1 else sq_q[0:1, 0:NK]
                    nc.vector.tensor_scalar(
                        dt_[:, 0:NK], src_ap, 1.0, None, op0=OP.mult
                    )
                    nc.vector.memset(dt_[:, NK:N], 0.0)
                    nc.sync.dma_start(out.ap(), dt_)

    nc.finalize()
    return nc


_CACHE: dict = {}


def _get_nc():
    key = (PATHS, ANCHOR)
    if key not in _CACHE:
        _CACHE[key] = build(PATHS, ANCHOR)
    return _CACHE[key]


def kernel(Z: np.ndarray, cls_tokens: np.ndarray) -> np.ndarray:
    assert Z.shape == (N, L, C) and cls_tokens.shape == (N, DC)
    Z = np.asarray(Z, dtype=np.float32).astype(np.float16)
    cls_tokens = np.ascontiguousarray(cls_tokens, dtype=np.float32)
    nc = _get_nc()
    in_maps = [
        {"Z": np.ascontiguousarray(np.roll(Z, -IPC * c, axis=0)), "cls_tokens": cls_tokens}
        for c in range(NCORES)
    ]
    res = bass_utils.run_bass_kernel_spmd(nc, in_maps, core_ids=list(range(NCORES)))
    return np.asarray(res.results[0]["out"], dtype=np.float32)


if __name__ == "__main__":
    rng = np.random.default_rng(0)
    Zv = rng.standard_normal((N, L, C), dtype=np.float32)
    cv = rng.standard_normal((N, DC), dtype=np.float32)
    print(kernel(Zv, cv)[:8])
